# revision 13
# baseline (speedup 1.0000x reference)
"""TENER-style MultiHeadedAttention TRN2 kernel (8 NeuronCores, SPMD).

Sharding v2: core c handles batch b = c//4 and head group g = c%4 (heads
[4g, 4g+4)), with ALL 1024 queries of its batch. Output projection produces a
partial [1024, 1024] out^T per core; a 4-core ReduceScatter (per batch) sums
the partials and leaves each core with a 256-row dm-slice of out^T.

Rationale vs v1 (query-sharded): eliminates the 4x-replicated V projection,
and widens every matmul moving dim to 512 (PE instruction overhead ~173ns is
paid per matmul, so wide streams matter more than anything else).

Key math (unchanged): TENER relative-position term folds into the score
contraction via angle addition:
  scores[j, s] = [k_j ; g_j] . [q_s ; a_sin(s) ; a_cos(s)]
with g_j = [sin(w j); cos(w j)], a_sin = qv_s cos(w s) + qv_c sin(w s),
a_cos = qv_c cos(w s) - qv_s sin(w s), qv = q + v_bias.
Softmax denominators come free via a ones-column appended per head to v.
"""

import math
import sys

sys.path.insert(0, "/opt/trn_rl_repo")

import numpy as np

B, S, D = 2, 1024, 1024
H, HD = 16, 64          # heads, head_dim
HALF = 32               # sin/cos half of head_dim
NC_ = 8                 # cores
HG = 4                  # heads per core
NQ = S                  # queries per core (all of its batch)
JT = S // 128           # 8 key tiles
FT = D // 128           # 8 feature tiles

_cache: dict = {}


def _rne_fp32r(a):
    """Round fp32 -> fp32r (1s+8e+11m) with round-to-nearest-even."""
    u = np.ascontiguousarray(a, dtype=np.float32).view(np.uint32)
    lsb = (u >> np.uint32(12)) & np.uint32(1)
    return ((u + np.uint32(0x7FF) + lsb) & np.uint32(0xFFFFF000)).view(np.float32)


# tabs column map (f32)
TC_C = 0          # cos(w s) table        [128, 1024]
TC_S = 1024       # sin(w s) table        [128, 1024]
TC_VBQ = 2048     # qv bias col per mt    [128, 2]
TC_BQ = 2050      # bq col per mt         [128, 2]
TC_ONES = 2052    # ones                  [128, 4]
TC_BO = 2056      # bo_eff/4 col per mt   [128, 8]
TBW = 2064


def _build_nc():
    import concourse.bacc as bacc
    import concourse.mybir as mybir
    from concourse import tile

    F32 = mybir.dt.float32
    F32R = mybir.dt.float32r
    ADD = mybir.AluOpType.add
    SUB = mybir.AluOpType.subtract
    MUL = mybir.AluOpType.mult
    EXP = mybir.ActivationFunctionType.Exp
    IDENT = mybir.ActivationFunctionType.Identity

    nc = bacc.Bacc("TRN2", target_bir_lowering=False, debug=False, num_devices=NC_)

    qpk_d = nc.dram_tensor("qpk", [D, NQ], F32R, kind="ExternalInput")
    wqk_d = nc.dram_tensor("wqk", [D, 256], F32R, kind="ExternalInput")
    vtk_d = nc.dram_tensor("vtk", [D, S], F32R, kind="ExternalInput")
    wvk_d = nc.dram_tensor("wvk", [D, 256], F32R, kind="ExternalInput")
    kgk_d = nc.dram_tensor("kgk", [HG * 128, S], F32R, kind="ExternalInput")
    wok_d = nc.dram_tensor("wok", [256, D], F32R, kind="ExternalInput")
    tabs_d = nc.dram_tensor("tabs", [128, TBW], F32, kind="ExternalInput")
    part_a = nc.dram_tensor("part_a", [512, NQ], F32, kind="Internal")
    part_b = nc.dram_tensor("part_b", [512, NQ], F32, kind="Internal")
    rs_a = nc.dram_tensor("rs_a", [128, NQ], F32, kind="Internal")
    rs_b = nc.dram_tensor("rs_b", [128, NQ], F32, kind="Internal")
    out_d = nc.dram_tensor("out", [256, NQ], F32, kind="ExternalOutput")

    import os as _os
    DBG = int(_os.environ.get("BASS_KERNEL_DEBUG", "0"))
    if DBG:
        dbg_catq = nc.dram_tensor("dbg_catq", [HG * 128, NQ], F32, kind="ExternalOutput")
        dbg_vv = nc.dram_tensor("dbg_vv", [128, HG * 65], F32, kind="ExternalOutput")
        dbg_xn = nc.dram_tensor("dbg_xn", [256, NQ], F32, kind="ExternalOutput")
        dbg_dr = nc.dram_tensor("dbg_dr", [2 * HG, NQ], F32, kind="ExternalOutput")
        dbg_rbs = nc.dram_tensor("dbg_rbs", [64, NQ], F32, kind="ExternalOutput")
        dbg_part = nc.dram_tensor("dbg_part", [512, NQ], F32, kind="ExternalOutput")

    RG = [[0, 1, 2, 3], [4, 5, 6, 7]]

    with tile.TileContext(nc, num_cores=NC_) as tc:
        with tc.tile_pool(name="persist", bufs=1) as pp, \
             tc.tile_pool(name="small", bufs=2) as sp:

            tabs = pp.tile([128, TBW], F32, tag="tabs")
            nc.scalar.dma_start(tabs[:], tabs_d.ap())

            kg = []
            for hh in range(HG):
                t = pp.tile([128, S], F32R, name=f"kg{hh}", tag=f"kg{hh}")
                nc.scalar.dma_start(t[:], kgk_d.ap()[hh * 128:(hh + 1) * 128, :])
                kg.append(t)

            catq = [pp.tile([128, NQ], F32R, name=f"catq{hh}", tag=f"catq{hh}")
                    for hh in range(HG)]
            vv = [pp.tile([128, HG * 65], F32R, name=f"vv{j}", tag=f"vv{j}")
                  for j in range(JT)]
            xn = [pp.tile([128, NQ], F32R, name=f"xn{c}", tag=f"xn{c}")
                  for c in range(2)]
            ebias = pp.tile([128, 1], F32, tag="ebias")
            nc.vector.memset(ebias[:], -25.0)
            dt = [pp.tile([1, NQ], F32, name=f"dt{hh}", tag=f"dt{hh}") for hh in range(HG)]
            rt = [pp.tile([1, NQ], F32, name=f"rt{hh}", tag=f"rt{hh}") for hh in range(HG)]

            # ---------- phase A: Q projection + V projection ----------
            with tc.tile_pool(name="wq", bufs=1) as wqp, \
                 tc.tile_pool(name="wv", bufs=1) as wvp, \
                 tc.tile_pool(name="qpkp", bufs=1) as qpkp, \
                 tc.tile_pool(name="vtkp", bufs=1) as vtkp, \
                 tc.tile_pool(name="qps", bufs=1, space="PSUM") as qps, \
                 tc.tile_pool(name="vps", bufs=1, space="PSUM") as vps, \
                 tc.tile_pool(name="rot", bufs=2) as rotp:

                wq_sb = []
                qpk_sb = []
                for c in range(FT):
                    tw = wqp.tile([128, 256], F32R, name=f"wq{c}", tag=f"wq{c}")
                    nc.sync.dma_start(tw[:], wqk_d.ap()[c * 128:(c + 1) * 128, :])
                    wq_sb.append(tw)
                    tq = qpkp.tile([128, NQ], F32R, name=f"qpk{c}", tag=f"qpk{c}")
                    nc.sync.dma_start(tq[:], qpk_d.ap()[c * 128:(c + 1) * 128, :])
                    qpk_sb.append(tq)

                wv_sb = []
                for c in range(FT):
                    tw = wvp.tile([128, 256], F32R, name=f"wv{c}", tag=f"wv{c}")
                    nc.gpsimd.dma_start(tw[:], wvk_d.ap()[c * 128:(c + 1) * 128, :])
                    wv_sb.append(tw)

                # Q: psum per (mt, nt) [128, 512], accumulate over c
                qpsum = [[qps.tile([128, 512], F32, name=f"qpsum{mt}{nt}", tag=f"qpsum{mt}{nt}")
                          for nt in range(2)] for mt in range(2)]
                for c in range(FT):
                    for mt in range(2):
                        for nt in range(2):
                            nc.tensor.matmul(
                                qpsum[mt][nt][:],
                                wq_sb[c][:, mt * 128:(mt + 1) * 128],
                                qpk_sb[c][:, nt * 512:(nt + 1) * 512],
                                start=(c == 0), stop=(c == FT - 1))

                # rotation + catq fill per M-tile (2 heads each)
                for mt in range(2):
                    hA, hB = 2 * mt, 2 * mt + 1
                    # top rows: plain q (+bq) via ACT copies
                    for nt in range(2):
                        nc.scalar.activation(
                            catq[hA][0:64, nt * 512:(nt + 1) * 512],
                            qpsum[mt][nt][0:64, :], IDENT,
                            bias=tabs[0:64, TC_BQ + mt:TC_BQ + mt + 1], scale=1.0)
                        nc.scalar.activation(
                            catq[hB][0:64, nt * 512:(nt + 1) * 512],
                            qpsum[mt][nt][64:128, :], IDENT,
                            bias=tabs[64:128, TC_BQ + mt:TC_BQ + mt + 1], scale=1.0)
                    qv = rotp.tile([128, NQ], F32, tag="qv")
                    for nt in range(2):
                        nc.vector.tensor_scalar(
                            out=qv[:, nt * 512:(nt + 1) * 512],
                            in0=qpsum[mt][nt][:],
                            scalar1=tabs[:, TC_VBQ + mt:TC_VBQ + mt + 1],
                            scalar2=None, op0=ADD)
                    t1 = rotp.tile([128, NQ], F32, tag="t1")
                    nc.vector.tensor_tensor(out=t1[:], in0=qv[:],
                                            in1=tabs[:, TC_C:TC_C + NQ], op=MUL)
                    t2 = rotp.tile([128, NQ], F32, tag="t2")
                    for gsrc, gdst in ((32, 0), (0, 32), (96, 64), (64, 96)):
                        nc.vector.tensor_tensor(
                            out=t2[gdst:gdst + 32, :],
                            in0=qv[gsrc:gsrc + 32, :],
                            in1=tabs[gsrc:gsrc + 32, TC_S:TC_S + NQ], op=MUL)
                    for hq, base in ((hA, 0), (hB, 64)):
                        nc.vector.tensor_tensor(
                            out=catq[hq][64:96, :], in0=t1[base:base + 32, :],
                            in1=t2[base:base + 32, :], op=ADD)
                        nc.vector.tensor_tensor(
                            out=catq[hq][96:128, :], in0=t1[base + 32:base + 64, :],
                            in1=t2[base + 32:base + 64, :], op=SUB)

                # V: psum pairs [128, 512] hold two jt's [keys, 256 vdims]
                vpsum = [vps.tile([128, 512], F32, name=f"vpsum{p}", tag=f"vpsum{p}") for p in range(4)]
                vtk_sb = []
                for c in range(FT):
                    tv = vtkp.tile([128, S], F32R, name=f"vtk{c}", tag=f"vtk{c}")
                    nc.gpsimd.dma_start(tv[:], vtk_d.ap()[c * 128:(c + 1) * 128, :])
                    vtk_sb.append(tv)
                for c in range(FT):
                    for jt in range(JT):
                        # start=True zeroes the WHOLE psum bank, so only the
                        # even jt of each bank pair may start; the odd jt
                        # accumulates onto the just-zeroed region.
                        nc.tensor.matmul(
                            vpsum[jt // 2][:, (jt % 2) * 256:(jt % 2) * 256 + 256],
                            vtk_sb[c][:, jt * 128:(jt + 1) * 128],
                            wv_sb[c][:],
                            start=(c == 0 and jt % 2 == 0), stop=(c == FT - 1),
                            skip_group_check=True)

                # vv fill: interleave [64 v | ones] per head
                for jt in range(JT):
                    nc.scalar.copy(
                        vv[jt][:].rearrange("p (h x) -> p h x", x=65)[:, :, 64:65],
                        tabs[:, TC_ONES:TC_ONES + HG].rearrange(
                            "p (h x) -> p h x", x=1))
                    dst = vv[jt][:].rearrange("p (h x) -> p h x", x=65)[:, :, 0:64]
                    src = vpsum[jt // 2][:, (jt % 2) * 256:(jt % 2) * 256 + 256]
                    nc.scalar.copy(dst, src.rearrange("p (h d) -> p h d", d=64))

            # ---------- phase B: attention ----------
            with tc.tile_pool(name="scps", bufs=2, space="PSUM") as scps, \
                 tc.tile_pool(name="xtps", bufs=2, space="PSUM") as xtps, \
                 tc.tile_pool(name="exp", bufs=4) as ep:
                for hh in range(HG):
                    xt = xtps.tile([65, NQ], F32, tag="xt")
                    for jt in range(JT):
                        sc = scps.tile([128, NQ], F32, tag="sc")
                        for nt in range(2):
                            nc.tensor.matmul(
                                sc[:, nt * 512:(nt + 1) * 512],
                                kg[hh][:, jt * 128:(jt + 1) * 128],
                                catq[hh][:, nt * 512:(nt + 1) * 512],
                                start=True, stop=True, skip_group_check=True)
                        ex = ep.tile([128, NQ], F32R, tag="ex")
                        for nt in range(2):
                            nc.scalar.activation(
                                ex[:, nt * 512:(nt + 1) * 512],
                                sc[:, nt * 512:(nt + 1) * 512],
                                EXP, bias=ebias[:], scale=1.0)
                        for nt in range(2):
                            nc.tensor.matmul(
                                xt[0:65, nt * 512:(nt + 1) * 512],
                                vv[jt][:, hh * 65:hh * 65 + 65],
                                ex[:, nt * 512:(nt + 1) * 512],
                                start=(jt == 0), stop=(jt == JT - 1),
                                skip_group_check=True)
                    # denom -> reciprocal -> broadcast -> normalize
                    nc.scalar.copy(dt[hh][:], xt[64:65, :])
                    nc.vector.reciprocal_approx_fast(out=rt[hh][:], in_=dt[hh][:])
                    rbs = sp.tile([64, NQ], F32, tag="rbs")
                    nc.gpsimd.partition_broadcast(rbs[:], rt[hh][:], channels=64)
                    nc.vector.tensor_tensor(
                        out=xn[hh // 2][(hh % 2) * 64:(hh % 2) * 64 + 64, :],
                        in0=xt[0:64, :], in1=rbs[:], op=MUL)
                    if DBG and hh == 0:
                        nc.sync.dma_start(dbg_rbs.ap(), rbs[:])

            if DBG:
                for hh in range(HG):
                    nc.sync.dma_start(
                        dbg_catq.ap()[hh * 128:(hh + 1) * 128, :],
                        catq[hh][:].bitcast(F32))
                nc.sync.dma_start(dbg_vv.ap(), vv[0][:].bitcast(F32))
                for c2 in range(2):
                    nc.sync.dma_start(
                        dbg_xn.ap()[c2 * 128:(c2 + 1) * 128, :],
                        xn[c2][:].bitcast(F32))
                for hh in range(HG):
                    nc.sync.dma_start(dbg_dr.ap()[hh:hh + 1, :], dt[hh][:])
                    nc.sync.dma_start(dbg_dr.ap()[HG + hh:HG + hh + 1, :], rt[hh][:])

            # ---------- phase C: output projection + ReduceScatter ----------
            with tc.tile_pool(name="wo", bufs=1) as wop, \
                 tc.tile_pool(name="ops", bufs=2, space="PSUM") as ops, \
                 tc.tile_pool(name="osb", bufs=2) as osbp:
                wo_sb = []
                for c2 in range(2):
                    tw = wop.tile([128, D], F32R, name=f"wo{c2}", tag=f"wo{c2}")
                    nc.scalar.dma_start(tw[:], wok_d.ap()[c2 * 128:(c2 + 1) * 128, :])
                    wo_sb.append(tw)

                for mt in range(8):
                    op = ops.tile([128, NQ], F32, tag="op")
                    for nt in range(2):
                        for c2 in range(2):
                            nc.tensor.matmul(
                                op[:, nt * 512:(nt + 1) * 512],
                                wo_sb[c2][:, mt * 128:(mt + 1) * 128],
                                xn[c2][:, nt * 512:(nt + 1) * 512],
                                start=(c2 == 0), stop=(c2 == 1),
                                skip_group_check=True)
                    os_ = osbp.tile([128, NQ], F32, tag="os")
                    nc.scalar.activation(
                        os_[:], op[:], IDENT,
                        bias=tabs[:, TC_BO + mt:TC_BO + mt + 1], scale=1.0)
                    tgt = part_a if mt < 4 else part_b
                    nc.sync.dma_start(
                        tgt.ap()[(mt % 4) * 128:(mt % 4) * 128 + 128, :], os_[:])
                    if DBG and mt < 4:
                        nc.gpsimd.dma_start(
                            dbg_part.ap()[mt * 128:(mt + 1) * 128, :], os_[:])
                    if mt == 3:
                        nc.gpsimd.collective_compute(
                            "ReduceScatter", ADD, replica_groups=RG,
                            ins=[part_a.ap()], outs=[rs_a.ap()])
                if True:
                    nc.gpsimd.collective_compute(
                        "ReduceScatter", ADD, replica_groups=RG,
                        ins=[part_b.ap()], outs=[rs_b.ap()])
                nc.sync.dma_start(out_d.ap()[0:128, :], rs_a.ap())
                nc.sync.dma_start(out_d.ap()[128:256, :], rs_b.ap())

    nc.finalize()
    return nc


def _host_pack(query, key, value, Wq, bq, Wv, bv, Wo, bo, v_bias):
    """Build the 8 per-core input maps."""
    r = _rne_fp32r
    w = np.exp(np.arange(HALF) * (-math.log(10000.0) / (HALF - 1))).astype(np.float64)

    WqT = np.ascontiguousarray(Wq.T)            # [1024 in, 1024 qdims]
    WvT = np.ascontiguousarray(Wv.T)            # [1024 in, 1024 vdims]
    WoT = np.ascontiguousarray(Wo.T)            # [1024 f, 1024 dm]
    bo_eff4 = (bo + Wo @ bv) / 4.0              # bv folds out via softmax sum=1

    # g table [64, S]
    j = np.arange(S, dtype=np.float64)
    gsin = np.sin(w[:, None] * j[None, :])
    gcos = np.cos(w[:, None] * j[None, :])
    g64 = np.concatenate([gsin, gcos], axis=0).astype(np.float32)  # [64, S]

    svals = np.arange(NQ, dtype=np.float64)[None, :]
    wrep = np.tile(w, 4)[:, None]               # [128, 1]
    ctab = np.cos(wrep * svals).astype(np.float32)    # [128, NQ]
    stab = np.sin(wrep * svals).astype(np.float32)

    qpks, vtks, kgks = [], [], []
    for b in range(B):
        qpks.append(r(query[b].T))
        vtks.append(r(value[b].T))
        kT = key[b].T
        kgb = np.empty((H * 128, S), np.float32)
        for h in range(H):
            kgb[h * 128:h * 128 + 64] = kT[h * 64:(h + 1) * 64]
            kgb[h * 128 + 64:h * 128 + 128] = g64
        kgks.append(r(kgb))

    in_maps = []
    for c in range(NC_):
        b, g = c // 4, c % 4
        tabs = np.zeros((128, TBW), np.float32)
        tabs[:, TC_C:TC_C + NQ] = ctab
        tabs[:, TC_S:TC_S + NQ] = stab
        for mt in range(2):
            hA = 4 * g + 2 * mt
            vb2 = np.concatenate([v_bias[hA], v_bias[hA + 1]])      # [128]
            bq2 = np.concatenate([bq[hA * 64:(hA + 1) * 64],
                                  bq[(hA + 1) * 64:(hA + 2) * 64]])
            tabs[:, TC_VBQ + mt] = vb2 + bq2
            tabs[:, TC_BQ + mt] = bq2
        tabs[:, TC_ONES:TC_ONES + 4] = 1.0
        tabs[0, TC_ONES:TC_ONES + 64] = 1.0  # ones row for rb broadcast
        for mt in range(8):
            tabs[:, TC_BO + mt] = bo_eff4[mt * 128:(mt + 1) * 128]

        in_maps.append({
            "qpk": qpks[b],
            "wqk": r(WqT[:, g * 256:(g + 1) * 256]),
            "vtk": vtks[b],
            "wvk": r(WvT[:, g * 256:(g + 1) * 256]),
            "kgk": kgks[b][g * 512:(g + 1) * 512, :],
            "wok": r(WoT[g * 256:(g + 1) * 256, :]),
            "tabs": tabs,
        })
    return in_maps


def kernel(query, key, value, mask, Wq, bq, Wv, bv, Wo, bo, v_bias):
    from concourse.bass_utils import run_bass_kernel_spmd

    query = np.asarray(query, np.float32)
    key = np.asarray(key, np.float32)
    value = np.asarray(value, np.float32)
    in_maps = _host_pack(query, key, value,
                         np.asarray(Wq, np.float32), np.asarray(bq, np.float32),
                         np.asarray(Wv, np.float32), np.asarray(bv, np.float32),
                         np.asarray(Wo, np.float32), np.asarray(bo, np.float32),
                         np.asarray(v_bias, np.float32))

    if "nc" not in _cache:
        _cache["nc"] = _build_nc()
    nc = _cache["nc"]

    import os
    if int(os.environ.get("BASS_KERNEL_TRACE", "0")):
        import importlib.util as _ilu
        if "antenv.axon_hooks" not in sys.modules:
            _spec = _ilu.spec_from_file_location(
                "antenv.axon_hooks", "/opt/trn_rl_repo/antenv/axon_hooks.py")
            _mod = _ilu.module_from_spec(_spec)
            _spec.loader.exec_module(_mod)
            sys.modules["antenv.axon_hooks"] = _mod
    res = run_bass_kernel_spmd(
        nc, in_maps, core_ids=list(range(NC_)),
        trace=bool(int(os.environ.get("BASS_KERNEL_TRACE", "0"))))
    _cache["last_result"] = res

    out = np.empty((B, S, D), np.float32)
    for c in range(NC_):
        b, g = c // 4, c % 4
        o = res.results[c]["out"]                 # [256, 1024] dm-slices of out^T
        out[b][:, 128 * g:128 * g + 128] = o[0:128, :].T
        out[b][:, 512 + 128 * g:512 + 128 * g + 128] = o[128:256, :].T
    return out


# revision 16
# speedup vs baseline: 1.1633x; 1.1633x over previous
"""TENER-style MultiHeadedAttention TRN2 kernel (8 NeuronCores, SPMD).

Sharding v2: core c handles batch b = c//4 and head group g = c%4 (heads
[4g, 4g+4)), with ALL 1024 queries of its batch. Output projection produces a
partial out^T per core; a 4-core ReduceScatter (per batch) sums the partials
and leaves each core with a 256-row dm-slice of out^T.

The attention + output projection are chunked over query halves so the first
half's ReduceScatter overlaps the second half's compute; partials travel as
bf16 to halve collective wire time.

Key math: TENER relative-position term folds into the score contraction via
angle addition:
  scores[j, s] = [k_j ; g_j] . [q_s ; a_sin(s) ; a_cos(s)]
with g_j = [sin(w j); cos(w j)], a_sin = qv_s cos(w s) + qv_c sin(w s),
a_cos = qv_c cos(w s) - qv_s sin(w s), qv = q + v_bias.
Softmax denominators come free via a ones-column appended per head to v.
"""

import math
import sys

sys.path.insert(0, "/opt/trn_rl_repo")

import numpy as np

B, S, D = 2, 1024, 1024
H, HD = 16, 64          # heads, head_dim
HALF = 32               # sin/cos half of head_dim
NC_ = 8                 # cores
HG = 4                  # heads per core
NQ = S                  # queries per core (all of its batch)
JT = S // 128           # 8 key tiles
FT = D // 128           # 8 feature tiles

_cache: dict = {}


def _rne_fp32r(a):
    """Round fp32 -> fp32r (1s+8e+11m) with round-to-nearest-even."""
    u = np.ascontiguousarray(a, dtype=np.float32).view(np.uint32)
    lsb = (u >> np.uint32(12)) & np.uint32(1)
    return ((u + np.uint32(0x7FF) + lsb) & np.uint32(0xFFFFF000)).view(np.float32)


# tabs column map (f32)
TC_C = 0          # cos(w s) table        [128, 1024]
TC_S = 1024       # sin(w s) table        [128, 1024]
TC_VBQ = 2048     # qv bias col per mt    [128, 2]
TC_BQ = 2050      # bq col per mt         [128, 2]
TC_ONES = 2052    # ones                  [128, 4]
TC_BO = 2056      # bo_eff/4 col per mt   [128, 8]
TBW = 2064


def _build_nc():
    import concourse.bacc as bacc
    import concourse.mybir as mybir
    from concourse import tile

    F32 = mybir.dt.float32
    F32R = mybir.dt.float32r
    BF16 = mybir.dt.bfloat16
    ADD = mybir.AluOpType.add
    SUB = mybir.AluOpType.subtract
    MUL = mybir.AluOpType.mult
    EXP = mybir.ActivationFunctionType.Exp
    IDENT = mybir.ActivationFunctionType.Identity

    nc = bacc.Bacc("TRN2", target_bir_lowering=False, debug=False, num_devices=NC_)

    qpk_d = nc.dram_tensor("qpk", [D, NQ], F32R, kind="ExternalInput")
    wqk_d = nc.dram_tensor("wqk", [D, 256], F32R, kind="ExternalInput")
    vtk_d = nc.dram_tensor("vtk", [D, S], F32R, kind="ExternalInput")
    wvk_d = nc.dram_tensor("wvk", [D, 256], F32R, kind="ExternalInput")
    kgk_d = nc.dram_tensor("kgk", [HG * 128, S], F32R, kind="ExternalInput")
    wok_d = nc.dram_tensor("wok", [256, D], F32R, kind="ExternalInput")
    tabs_d = nc.dram_tensor("tabs", [128, TBW], F32, kind="ExternalInput")
    part_q = [nc.dram_tensor(f"part_q{i}", [D, 512], BF16, kind="Internal")
              for i in range(2)]
    rs_q = [nc.dram_tensor(f"rs_q{i}", [256, 512], BF16, kind="Internal")
            for i in range(2)]
    out_d = nc.dram_tensor("out", [256, NQ], BF16, kind="ExternalOutput")

    RG = [[0, 1, 2, 3], [4, 5, 6, 7]]

    with tile.TileContext(nc, num_cores=NC_) as tc:
        with tc.tile_pool(name="persist", bufs=1) as pp, \
             tc.tile_pool(name="small", bufs=2) as sp:

            tabs = pp.tile([128, TBW], F32, tag="tabs")
            nc.sync.dma_start(tabs[:], tabs_d.ap())

            kg = []
            for hh in range(HG):
                t = pp.tile([128, S], F32R, name=f"kg{hh}", tag=f"kg{hh}")
                kg.append(t)

            wo_sb = [pp.tile([128, D], F32R, name=f"wo{c2}", tag=f"wo{c2}")
                     for c2 in range(2)]

            catq = [pp.tile([128, NQ], F32R, name=f"catq{hh}", tag=f"catq{hh}")
                    for hh in range(HG)]
            vv = [pp.tile([128, HG * 65], F32R, name=f"vv{j}", tag=f"vv{j}")
                  for j in range(JT)]
            xn = [pp.tile([128, NQ], F32R, name=f"xn{c}", tag=f"xn{c}")
                  for c in range(2)]
            ebias = pp.tile([128, 1], F32, tag="ebias")
            nc.vector.memset(ebias[:], -25.0)
            dt = [pp.tile([1, NQ], F32, name=f"dt{hh}", tag=f"dt{hh}")
                  for hh in range(HG)]
            rt = [pp.tile([1, NQ], F32, name=f"rt{hh}", tag=f"rt{hh}")
                  for hh in range(HG)]

            # ---------- phase A: Q projection + V projection ----------
            with tc.tile_pool(name="wq", bufs=1) as wqp, \
                 tc.tile_pool(name="wv", bufs=1) as wvp, \
                 tc.tile_pool(name="qpkp", bufs=1) as qpkp, \
                 tc.tile_pool(name="vtkp", bufs=1) as vtkp, \
                 tc.tile_pool(name="qps", bufs=1, space="PSUM") as qps, \
                 tc.tile_pool(name="vps", bufs=1, space="PSUM") as vps, \
                 tc.tile_pool(name="rot", bufs=2) as rotp:

                wq_sb = []
                qpk_sb = []
                for c in range(FT):
                    tw = wqp.tile([128, 256], F32R, name=f"wq{c}", tag=f"wq{c}")
                    nc.scalar.dma_start(tw[:], wqk_d.ap()[c * 128:(c + 1) * 128, :])
                    wq_sb.append(tw)
                    tq = qpkp.tile([128, NQ], F32R, name=f"qpk{c}", tag=f"qpk{c}")
                    eng = nc.sync if c % 2 == 0 else nc.scalar
                    eng.dma_start(tq[:], qpk_d.ap()[c * 128:(c + 1) * 128, :])
                    qpk_sb.append(tq)

                wv_sb = []
                vtk_sb = []
                for c in range(FT):
                    tw = wvp.tile([128, 256], F32R, name=f"wv{c}", tag=f"wv{c}")
                    nc.sync.dma_start(tw[:], wvk_d.ap()[c * 128:(c + 1) * 128, :])
                    wv_sb.append(tw)
                    tv = vtkp.tile([128, S], F32R, name=f"vtk{c}", tag=f"vtk{c}")
                    nc.gpsimd.dma_start(tv[:], vtk_d.ap()[c * 128:(c + 1) * 128, :])
                    vtk_sb.append(tv)
                for hh in range(HG):
                    nc.gpsimd.dma_start(kg[hh][:],
                                        kgk_d.ap()[hh * 128:(hh + 1) * 128, :])
                for c2 in range(2):
                    nc.gpsimd.dma_start(wo_sb[c2][:],
                                        wok_d.ap()[c2 * 128:(c2 + 1) * 128, :])

                # Q: psum per (mt, nt) [128, 512], accumulate over c
                qpsum = [[qps.tile([128, 512], F32, name=f"qpsum{mt}{nt}",
                                   tag=f"qpsum{mt}{nt}")
                          for nt in range(2)] for mt in range(2)]
                for c in range(FT):
                    for mt in range(2):
                        for nt in range(2):
                            nc.tensor.matmul(
                                qpsum[mt][nt][:],
                                wq_sb[c][:, mt * 128:(mt + 1) * 128],
                                qpk_sb[c][:, nt * 512:(nt + 1) * 512],
                                start=(c == 0), stop=(c == FT - 1))

                # rotation + catq fill per M-tile (2 heads each)
                for mt in range(2):
                    hA, hB = 2 * mt, 2 * mt + 1
                    # top rows: plain q (+bq) via ACT copies
                    for nt in range(2):
                        nc.scalar.activation(
                            catq[hA][0:64, nt * 512:(nt + 1) * 512],
                            qpsum[mt][nt][0:64, :], IDENT,
                            bias=tabs[0:64, TC_BQ + mt:TC_BQ + mt + 1], scale=1.0)
                        nc.scalar.activation(
                            catq[hB][0:64, nt * 512:(nt + 1) * 512],
                            qpsum[mt][nt][64:128, :], IDENT,
                            bias=tabs[64:128, TC_BQ + mt:TC_BQ + mt + 1], scale=1.0)
                    qv = rotp.tile([128, NQ], F32, tag="qv")
                    for nt in range(2):
                        nc.vector.tensor_scalar(
                            out=qv[:, nt * 512:(nt + 1) * 512],
                            in0=qpsum[mt][nt][:],
                            scalar1=tabs[:, TC_VBQ + mt:TC_VBQ + mt + 1],
                            scalar2=None, op0=ADD)
                    t1 = rotp.tile([128, NQ], F32, tag="t1")
                    nc.vector.tensor_tensor(out=t1[:], in0=qv[:],
                                            in1=tabs[:, TC_C:TC_C + NQ], op=MUL)
                    t2 = rotp.tile([128, NQ], F32, tag="t2")
                    for gsrc, gdst in ((32, 0), (0, 32), (96, 64), (64, 96)):
                        nc.vector.tensor_tensor(
                            out=t2[gdst:gdst + 32, :],
                            in0=qv[gsrc:gsrc + 32, :],
                            in1=tabs[gsrc:gsrc + 32, TC_S:TC_S + NQ], op=MUL)
                    for hq, base in ((hA, 0), (hB, 64)):
                        nc.vector.tensor_tensor(
                            out=catq[hq][64:96, :], in0=t1[base:base + 32, :],
                            in1=t2[base:base + 32, :], op=ADD)
                        nc.vector.tensor_tensor(
                            out=catq[hq][96:128, :], in0=t1[base + 32:base + 64, :],
                            in1=t2[base + 32:base + 64, :], op=SUB)

                # V: psum pairs [128, 512] hold two jt's [keys, 256 vdims]
                vpsum = [vps.tile([128, 512], F32, name=f"vpsum{p}", tag=f"vpsum{p}")
                         for p in range(4)]
                for c in range(FT):
                    for jt in range(JT):
                        # start=True zeroes the WHOLE psum bank, so only the
                        # even jt of each bank pair may start; the odd jt
                        # accumulates onto the just-zeroed region.
                        nc.tensor.matmul(
                            vpsum[jt // 2][:, (jt % 2) * 256:(jt % 2) * 256 + 256],
                            vtk_sb[c][:, jt * 128:(jt + 1) * 128],
                            wv_sb[c][:],
                            start=(c == 0 and jt % 2 == 0), stop=(c == FT - 1),
                            skip_group_check=True)

                # vv fill: interleave [64 v | ones] per head
                for jt in range(JT):
                    nc.scalar.copy(
                        vv[jt][:].rearrange("p (h x) -> p h x", x=65)[:, :, 64:65],
                        tabs[:, TC_ONES:TC_ONES + HG].rearrange(
                            "p (h x) -> p h x", x=1))
                    dst = vv[jt][:].rearrange("p (h x) -> p h x", x=65)[:, :, 0:64]
                    src = vpsum[jt // 2][:, (jt % 2) * 256:(jt % 2) * 256 + 256]
                    nc.scalar.copy(dst, src.rearrange("p (h d) -> p h d", d=64))

            # ---------- phase B/C: attention + out-proj, chunked by q-half ----
            with tc.tile_pool(name="scps", bufs=3, space="PSUM") as scps, \
                 tc.tile_pool(name="xtps", bufs=2, space="PSUM") as xtps, \
                 tc.tile_pool(name="ops", bufs=2, space="PSUM") as ops, \
                 tc.tile_pool(name="exp", bufs=4) as ep, \
                 tc.tile_pool(name="osb", bufs=2) as osbp:
                for qh in range(2):
                    q0 = qh * 512
                    for hh in range(HG):
                        xt = xtps.tile([65, 512], F32, tag="xt")
                        scs = []
                        exs = []

                        def do_scores(jt):
                            sc = scps.tile([128, 512], F32, tag="sc")
                            nc.tensor.matmul(
                                sc[:], kg[hh][:, jt * 128:(jt + 1) * 128],
                                catq[hh][:, q0:q0 + 512],
                                start=True, stop=True, skip_group_check=True)
                            scs.append(sc)

                        def do_exp(jt):
                            ex = ep.tile([128, 512], F32R, tag="ex")
                            nc.scalar.activation(ex[:], scs[jt][:], EXP,
                                                 bias=ebias[:], scale=1.0)
                            exs.append(ex)

                        def do_attnv(jt):
                            nc.tensor.matmul(
                                xt[0:65, :], vv[jt][:, hh * 65:hh * 65 + 65],
                                exs[jt][:],
                                start=(jt == 0), stop=(jt == JT - 1),
                                skip_group_check=True)

                        # software pipeline: keep PE one step ahead of ACT
                        do_scores(0)
                        do_exp(0)
                        for jt in range(1, JT):
                            do_scores(jt)
                            do_attnv(jt - 1)
                            do_exp(jt)
                        do_attnv(JT - 1)

                        # denom -> reciprocal -> broadcast -> normalize
                        nc.scalar.copy(dt[hh][0:1, q0:q0 + 512], xt[64:65, :])
                        nc.vector.reciprocal_approx_fast(
                            out=rt[hh][0:1, q0:q0 + 512],
                            in_=dt[hh][0:1, q0:q0 + 512])
                        rbs = sp.tile([64, 512], F32, tag="rbs")
                        nc.gpsimd.partition_broadcast(
                            rbs[:], rt[hh][0:1, q0:q0 + 512], channels=64)
                        nc.vector.tensor_tensor(
                            out=xn[hh // 2][(hh % 2) * 64:(hh % 2) * 64 + 64,
                                            q0:q0 + 512],
                            in0=xt[0:64, :], in1=rbs[:], op=MUL)

                    # out-proj for this q-half
                    for mt in range(8):
                        op = ops.tile([128, 512], F32, tag="op")
                        for c2 in range(2):
                            nc.tensor.matmul(
                                op[:], wo_sb[c2][:, mt * 128:(mt + 1) * 128],
                                xn[c2][:, q0:q0 + 512],
                                start=(c2 == 0), stop=(c2 == 1),
                                skip_group_check=True)
                        os_ = osbp.tile([128, 512], BF16, tag="os")
                        nc.scalar.activation(
                            os_[:], op[:], IDENT,
                            bias=tabs[:, TC_BO + mt:TC_BO + mt + 1], scale=1.0)
                        nc.sync.dma_start(
                            part_q[qh].ap()[mt * 128:(mt + 1) * 128, :], os_[:])
                    nc.gpsimd.collective_compute(
                        "ReduceScatter", ADD, replica_groups=RG,
                        ins=[part_q[qh].ap()], outs=[rs_q[qh].ap()])
                    nc.sync.dma_start(out_d.ap()[:, q0:q0 + 512], rs_q[qh].ap())

    nc.finalize()
    return nc


def _host_pack(query, key, value, Wq, bq, Wv, bv, Wo, bo, v_bias):
    """Build the 8 per-core input maps."""
    r = _rne_fp32r
    w = np.exp(np.arange(HALF) * (-math.log(10000.0) / (HALF - 1))).astype(np.float64)

    WqT = np.ascontiguousarray(Wq.T)            # [1024 in, 1024 qdims]
    WvT = np.ascontiguousarray(Wv.T)            # [1024 in, 1024 vdims]
    WoT = np.ascontiguousarray(Wo.T)            # [1024 f, 1024 dm]
    bo_eff4 = (bo + Wo @ bv) / 4.0              # bv folds out via softmax sum=1

    # g table [64, S]
    j = np.arange(S, dtype=np.float64)
    gsin = np.sin(w[:, None] * j[None, :])
    gcos = np.cos(w[:, None] * j[None, :])
    g64 = np.concatenate([gsin, gcos], axis=0).astype(np.float32)  # [64, S]

    svals = np.arange(NQ, dtype=np.float64)[None, :]
    wrep = np.tile(w, 4)[:, None]               # [128, 1]
    ctab = np.cos(wrep * svals).astype(np.float32)    # [128, NQ]
    stab = np.sin(wrep * svals).astype(np.float32)

    qpks, vtks, kgks = [], [], []
    for b in range(B):
        qpks.append(r(query[b].T))
        vtks.append(r(value[b].T))
        kT = key[b].T
        kgb = np.empty((H * 128, S), np.float32)
        for h in range(H):
            kgb[h * 128:h * 128 + 64] = kT[h * 64:(h + 1) * 64]
            kgb[h * 128 + 64:h * 128 + 128] = g64
        kgks.append(r(kgb))

    in_maps = []
    for c in range(NC_):
        b, g = c // 4, c % 4
        tabs = np.zeros((128, TBW), np.float32)
        tabs[:, TC_C:TC_C + NQ] = ctab
        tabs[:, TC_S:TC_S + NQ] = stab
        for mt in range(2):
            hA = 4 * g + 2 * mt
            vb2 = np.concatenate([v_bias[hA], v_bias[hA + 1]])      # [128]
            bq2 = np.concatenate([bq[hA * 64:(hA + 1) * 64],
                                  bq[(hA + 1) * 64:(hA + 2) * 64]])
            tabs[:, TC_VBQ + mt] = vb2 + bq2
            tabs[:, TC_BQ + mt] = bq2
        tabs[:, TC_ONES:TC_ONES + 4] = 1.0
        for mt in range(8):
            tabs[:, TC_BO + mt] = bo_eff4[mt * 128:(mt + 1) * 128]

        in_maps.append({
            "qpk": qpks[b],
            "wqk": r(WqT[:, g * 256:(g + 1) * 256]),
            "vtk": vtks[b],
            "wvk": r(WvT[:, g * 256:(g + 1) * 256]),
            "kgk": kgks[b][g * 512:(g + 1) * 512, :],
            "wok": r(WoT[g * 256:(g + 1) * 256, :]),
            "tabs": tabs,
        })
    return in_maps


def kernel(query, key, value, mask, Wq, bq, Wv, bv, Wo, bo, v_bias):
    from concourse.bass_utils import run_bass_kernel_spmd

    query = np.asarray(query, np.float32)
    key = np.asarray(key, np.float32)
    value = np.asarray(value, np.float32)
    in_maps = _host_pack(query, key, value,
                         np.asarray(Wq, np.float32), np.asarray(bq, np.float32),
                         np.asarray(Wv, np.float32), np.asarray(bv, np.float32),
                         np.asarray(Wo, np.float32), np.asarray(bo, np.float32),
                         np.asarray(v_bias, np.float32))

    if "nc" not in _cache:
        _cache["nc"] = _build_nc()
    nc = _cache["nc"]

    import os
    if int(os.environ.get("BASS_KERNEL_TRACE", "0")):
        import importlib.util as _ilu
        if "antenv.axon_hooks" not in sys.modules:
            _spec = _ilu.spec_from_file_location(
                "antenv.axon_hooks", "/opt/trn_rl_repo/antenv/axon_hooks.py")
            _mod = _ilu.module_from_spec(_spec)
            _spec.loader.exec_module(_mod)
            sys.modules["antenv.axon_hooks"] = _mod
    res = run_bass_kernel_spmd(
        nc, in_maps, core_ids=list(range(NC_)),
        trace=bool(int(os.environ.get("BASS_KERNEL_TRACE", "0"))))
    _cache["last_result"] = res

    out = np.empty((B, S, D), np.float32)
    for c in range(NC_):
        b, g = c // 4, c % 4
        o = np.asarray(res.results[c]["out"], np.float32)  # [256, 1024] out^T dm-slice
        out[b][:, 256 * g:256 * g + 256] = o.T
    return out
